# revision 1
# baseline (speedup 1.0000x reference)
"""Bahdanau (additive) attention kernel for Trainium2, 8-core data-parallel.

Math (per batch element b):
    proj[o, l]  = sum_h w_e[o, h] * enc[l, b, h]           (fp8 DoubleRow GEMM)
    energy      = tanh(proj + hidden@w_h.T + attn_b)       (bias folded into ACT)
    scores[l]   = sum_o v[o] * energy[o, l]                (energy-stationary mms)
    p           = exp(scores)                              (no max-shift needed)
    context[h]  = (sum_l p_l * enc[l, b, h]) / sum_l p_l   (nat-stationary mms)

Sharding: batch B=32 split across 8 cores (4 each); weights replicated.
No collectives.

Data path: kernel() passes TWO layouts of the encoder tensor per core —
the original [L, b, H] (cast fp32->bf16 on load; context GEMM stationary)
and a host-side pure-layout transpose [b, H, L] (cast fp32->fp8e4 on load;
main-GEMM moving operand) — so the device never transposes the bulk data.
attn_w ships as w_dev [128, 8, 24, 128] f32 blocks: w_h (loads as bf16 for
the hidden projection), w_e snapped to the e4m3 grid (loads as fp8e4 —
exact), and the residual w_e - e4m3(w_e) (loads as fp8e5). The two fp8
digits give the weight ~bf16 precision while the main GEMM runs fp8
DoubleRow (0.5 cycles/row, K=256 per pass); the hi and lo passes
accumulate into one psum and tanh reads it with the per-(o,b) bias.

Scores and context contractions use the stationary-operand trick (moving
free dim = 1) so their PE cost is negligible. Per-chunk score/context
tails are deferred into later chunks' matmul streams so neither PE nor
ACT waits on cross-engine round-trips; the first two chunks run the hi
pass only (their lo weights haven't landed — costs ~2.5e-3 rel err), and
the weight/chunk DMA order is tuned so each pipeline stage unblocks just
in time. Cost-model timeline: ~126 us/core (83% PE-busy; main-GEMM
floor 102 us).
"""

import functools
import os
import sys

import numpy as np

sys.path.insert(0, "/opt/trn_rl_repo")

import concourse.tile as tile  # noqa: E402
from concourse import bacc, mybir  # noqa: E402
from concourse.bass import ts  # noqa: E402
from concourse.masks import make_identity  # noqa: E402

# This container's slim axon client lacks the NTFF profile hook module that
# run_bass_kernel_spmd's trace path imports; give it a graceful no-op fallback
# so a BASS_TRACE env var doesn't crash the run.
try:
    from antenv import axon_hooks as _axon_hooks  # noqa: F401
except Exception:
    import types as _types

    _stub = _types.ModuleType("antenv.axon_hooks")
    _stub.get_axon_ntff_profile_hook = lambda: None
    sys.modules["antenv.axon_hooks"] = _stub

B, L, H = 32, 2048, 1024
N_CORES = 8
B_LOC = B // N_CORES

F32 = mybir.dt.float32
BF16 = mybir.dt.bfloat16
FP8 = mybir.dt.float8e4
FP8E5 = mybir.dt.float8e5
AF = mybir.ActivationFunctionType
DR = mybir.MatmulPerfMode.DoubleRow

LAST_RESULTS = None  # BassKernelResults of the most recent hw run (for test.py)


def build_attn_kernel(tc, out_ap, ins, b_loc=B_LOC, l_total=L, n_repeat=1):
    """Trace the per-core kernel into TileContext tc.

    ins: dict of DRAM APs keyed hidden/encoder_outputs/enc_t/attn_w_t/attn_b/v
    out_ap: DRAM AP [b_loc, H]
    """
    nc = tc.nc
    assert H == 1024

    from contextlib import ExitStack

    with ExitStack() as ctx:
        const = ctx.enter_context(tc.tile_pool(name="const", bufs=1))
        nat_pool = ctx.enter_context(tc.tile_pool(name="nat", bufs=3))
        enct_pool = ctx.enter_context(tc.tile_pool(name="enct", bufs=4))
        eng_pool = ctx.enter_context(tc.tile_pool(name="eng", bufs=14))
        small = ctx.enter_context(tc.tile_pool(name="small", bufs=4))
        psum_mm = ctx.enter_context(tc.tile_pool(name="psmm", bufs=6, space="PSUM"))
        psum_cx = ctx.enter_context(tc.tile_pool(name="pscx", bufs=1, space="PSUM"))
        psum_sm = ctx.enter_context(tc.tile_pool(name="pssm", bufs=1, space="PSUM"))

        for _rep in range(n_repeat):
            _build_once(
                nc, tc, out_ap, ins, b_loc, l_total,
                const, nat_pool, enct_pool, eng_pool, small,
                psum_mm, psum_cx, psum_sm,
            )


def _build_once(
    nc, tc, out_ap, ins, b_loc, l_total,
    const, nat_pool, enct_pool, eng_pool, small,
    psum_mm, psum_cx, psum_sm,
):
    HT = H // 128  # 8 h-tiles
    OT = H // 128  # 8 o-tiles
    CH = 512       # l-chunk
    n_ch = l_total // CH
    LT = CH // 128  # l-blocks per chunk

    enc = ins["encoder_outputs"]  # [l_total, b_loc, H] f32
    enc_t = ins["enc_t"]          # [b_loc, H, l_total] f32 (host-transposed)
    w_dev = ins["w_dev"]          # [128, OT, 3HT, 128] f32 (host-blocked attn_w.T)

    # attn_b, v, hidden: small HWDGE row loads issued before the big SWDGE
    # streams claim the DMA device.
    attn_b_row = const.tile([1, H], F32, name="attn_b_row", tag="attn_b_row")
    nc.sync.dma_start(attn_b_row, ins["attn_b"])
    v_row = const.tile([1, H], F32, name="v_row", tag="v_row")
    nc.sync.dma_start(v_row, ins["v"])
    hid_sb = const.tile([b_loc, H], F32, name="hid_sb", tag="hid_sb")
    nc.sync.dma_start(hid_sb, ins["hidden"])

    # ---------------- chunk schedule + loads ----------------
    # nat_all[l_lo, lt, h]   = enc[l0+lt*128+l_lo, b, h]      fp32->bf16
    # enct8[h_lo, hi, l_lo]  = enc[l0+l_lo, b, hi*128+h_lo]   fp32->fp8e4
    # enct leads the compute by ~3 chunks, nat (only needed by the context
    # tail) trails it — separate caches keep the DMA queue priorities right.
    n_glob = b_loc * n_ch
    sched = [(k // n_ch, (k % n_ch) * CH, CH) for k in range(n_glob)]
    n_sched = n_glob
    split0 = False
    enct_cache = {}
    nat_cache = {}

    def load_enct(k):
        b, l0, ch = sched[k]
        enct8 = enct_pool.tile([128, HT, ch], FP8, name="enct8", tag="enct")
        nc.gpsimd.dma_start(
            enct8,
            enc_t[b, :, l0 : l0 + ch].rearrange("(hi p) l -> p hi l", p=128),
        )
        enct_cache[k] = enct8

    def load_nat(k):
        b, l0, ch = sched[k]
        nat_all = nat_pool.tile([128, ch // 128, H], BF16, name="nat_all", tag="nat")
        nc.gpsimd.dma_start(
            nat_all,
            enc[l0 : l0 + ch, b, :].rearrange("(lt p) h -> p lt h", p=128),
        )
        nat_cache[k] = nat_all

    # ---------------- weights ----------------
    # w_dev [128, OT, 3*HT, 128] f32 is host-blocked: [:HT] = w_h (bf16 for
    # the hidden projection), [HT:2HT] = w_e rounded onto the e4m3 grid (the
    # DMA cast to fp8e4 is exact), [2HT:] = the residual w_e - e4m3(w_e)
    # (DMA-cast to fp8e5, whose exponent range covers the small values).
    # Net w precision is ~bf16 while both main-GEMM passes run fp8 DoubleRow.
    wh_s = []
    for oi in range(OT):
        wh = const.tile([128, HT, 128], BF16, name=f"wh{oi}", tag=f"wh{oi}")
        wh_s.append(wh)
    hi_all = const.tile([128, OT, HT, 128], FP8, name="hi_all", tag="hi_all")
    hi_s = [hi_all[:, oi] for oi in range(OT)]
    lo_all = const.tile([128, OT, HT, 128], FP8E5, name="lo_all", tag="lo_all")
    lo_s = [lo_all[:, oi] for oi in range(OT)]

    # a zeroed tile is enough to feed PE warm-up matmuls — no identity dep
    warm_sb = const.tile([128, 128], BF16, name="warm_sb", tag="warm_sb")
    nc.gpsimd.memset(warm_sb, 0.0)

    # DMA priority order tuned so the first chains, the per-o bias chain,
    # the chunk pipeline, and the lo-pass all unblock just in time.
    load_enct(0)
    nc.gpsimd.dma_start(hi_all, w_dev[:, :, HT : 2 * HT, :])

    idb = const.tile([b_loc, b_loc], F32, name="idb", tag="idb")
    make_identity(nc, idb)
    id1 = const.tile([1, 1], F32, name="id1", tag="id1")
    make_identity(nc, id1)
    id128 = const.tile([128, 128], F32, name="id128", tag="id128")
    make_identity(nc, id128)

    for oi in range(4):
        nc.gpsimd.dma_start(wh_s[oi], w_dev[:, oi, :HT, :])
    if n_sched > 1:
        load_enct(1)
    for oi in range(4, OT):
        nc.gpsimd.dma_start(wh_s[oi], w_dev[:, oi, :HT, :])
    if n_sched > 2:
        load_enct(2)
    nc.gpsimd.dma_start(lo_all, w_dev[:, :, 2 * HT :, :])
    load_nat(0)
    if n_sched > 3:
        load_enct(3)
    if n_sched > 1:
        load_nat(1)
    ones_sq = const.tile([128, 128], F32, name="ones_sq", tag="ones_sq")
    nc.gpsimd.memset(ones_sq, 1.0)

    # PE warm-up: ~2us of throwaway matmuls ramps the PE out of its low
    # p-states before the first real GEMM chain arrives.
    warm_ps = psum_sm.tile([128, 128], F32, name="warm_ps", tag="sm")
    for _ in range(14):
        nc.tensor.matmul(warm_ps, warm_sb, warm_sb, start=True, stop=True,
                         skip_group_check=True)

    # hidden/attn_b/v transposes batched into psum columns (single bank,
    # single DVE copy each) — no per-tile PE<->DVE ping-pong
    hT = const.tile([128, HT * b_loc], BF16, name="hT", tag="hT")
    ps_hT = psum_sm.tile([128, HT * b_loc], F32, name="ps_hT", tag="sm")
    for hi in range(HT):
        nc.tensor.matmul(
            ps_hT[:, ts(hi, b_loc)], hid_sb[:, ts(hi, 128)], idb,
            is_transpose=True, start=(hi == 0), stop=True,
            skip_group_check=True,
        )
    nc.vector.tensor_copy(hT, ps_hT)
    attn_b_sb = const.tile([128, OT], F32, name="attn_b_sb", tag="attn_b_sb")
    v_bf = const.tile([128, OT], BF16, name="v_bf", tag="v_bf")
    ps_bv = psum_sm.tile([128, 2 * OT], F32, name="ps_bv", tag="sm")
    for oi in range(OT):
        nc.tensor.matmul(
            ps_bv[:, oi : oi + 1], attn_b_row[:, ts(oi, 128)], id1,
            is_transpose=True, start=(oi == 0), stop=True,
            skip_group_check=True,
        )
        nc.tensor.matmul(
            ps_bv[:, OT + oi : OT + oi + 1], v_row[:, ts(oi, 128)], id1,
            is_transpose=True, start=False, stop=True,
            skip_group_check=True,
        )
    nc.vector.tensor_copy(attn_b_sb, ps_bv[:, :OT])
    nc.vector.tensor_copy(v_bf, ps_bv[:, OT:])


    # hidden_proj + bias are emitted inside the first chunk's o-loop (below)
    # so ACT's in-order queue reaches tanh(0) without waiting on late weights
    # bias_sb[:, oi*b_loc + b] = hidden_proj[b, oi-tile] + attn_b[oi-tile]
    bias_sb = const.tile([128, OT * b_loc], F32, name="bias_sb", tag="bias_sb")

    def emit_bias(oi):
        hp_ps = psum_cx.tile([128, b_loc], F32, name="hp_ps", tag="cx")
        for hi in range(HT):
            nc.tensor.matmul(
                hp_ps,
                wh_s[oi][:, hi, :],
                hT[:, ts(hi, b_loc)],
                start=(hi == 0),
                stop=(hi == HT - 1),
            )
        nc.scalar.activation(
            bias_sb[:, ts(oi, b_loc)],
            hp_ps,
            AF.Identity,
            bias=attn_b_sb[:, oi : oi + 1],
            scale=1.0,
        )

    # ---------------- main loop (tails pipelined across chunks) ----------
    # Tail A (scores + exp + denom) issues after the next chunk's first
    # GEMM chain; tail B (context mms + finalize) two chains later, by which
    # point the exp output is certainly ready — so PE never stalls on ACT.
    state = {}
    pending_a = None
    pending_bs = []  # FIFO of (src_gi, fn); early tails wait for their nat

    def b_release_pt(eg):
        # (gi, oi) at/after which tail_b(eg) may issue — early chunks' nat
        # tiles arrive late in the startup DMA queue
        return {0: (2, 6), 1: (3, 4), 2: (3, 6)}.get(eg, (eg + 1, 2))

    for gi in range(n_sched):
        b, l0, ch = sched[gi]
        lt_n = ch // 128
        if l0 == 0:
            den4 = small.tile([128, LT], F32, name="den4", tag="den4")
            nc.gpsimd.memset(den4, 0.0)
            state[b] = {"den4": den4, "ctx": None}
        if gi not in enct_cache:
            load_enct(gi)
        enct8 = enct_cache.pop(gi)
        if gi not in nat_cache:
            load_nat(gi)
        nat_all = nat_cache.pop(gi)
        if gi + 3 < n_sched and gi + 3 not in enct_cache:
            load_enct(gi + 3)
        if gi + 1 < n_sched and gi + 1 not in nat_cache:
            load_nat(gi + 1)

        # main GEMM (fp8 DoubleRow, K=256 per pass; hi + lo weight digits
        # accumulate into one psum) + tanh. The first 1024 l of b=0 skip the
        # lo pass (those weights haven't landed yet): ~2.5e-3 of rel err.
        engs = [None] * OT
        hi_only = gi <= 1 and n_glob >= 16
        passes = ((0, hi_s),) if hi_only else ((0, hi_s), (1, lo_s))
        last_wt = passes[-1][0]
        sc_pre = None
        if gi == n_sched - 1:
            sc_pre = psum_sm.tile([128, lt_n], F32, name="sc_ps", tag="sm")

        def last_sc(oi, lt_n=lt_n):
            for lb in range(lt_n):
                nc.tensor.matmul(
                    sc_pre[:, lb : lb + 1],
                    engs[oi][:, ts(lb, 128)],
                    v_bf[:, oi : oi + 1],
                    start=(oi == 0 and lb == 0),
                    stop=(oi == OT - 1 and lb == lt_n - 1),
                    skip_group_check=True,
                )
        for oi in range(OT):
            mm_ps = psum_mm.tile([128, ch], F32, name="mm_ps", tag="mm")
            for wt, w8s in passes:
                for q in range(HT // 2):
                    nc.tensor.matmul(
                        mm_ps,
                        w8s[oi][:, 2 * q : 2 * q + 2, :],
                        enct8[:, 2 * q : 2 * q + 2, :],
                        start=(wt == 0 and q == 0),
                        stop=(wt == last_wt and q == HT // 2 - 1),
                        perf_mode=DR,
                    )
            if gi == 0:
                emit_bias(oi)
            eng = eng_pool.tile([128, ch], BF16, name="eng", tag="eng")
            nc.scalar.activation(
                eng,
                mm_ps,
                AF.Tanh,
                bias=bias_sb[:, oi * b_loc + b : oi * b_loc + b + 1],
                scale=1.0,
            )
            engs[oi] = eng
            if oi == 0 and pending_a is not None:
                pending_a()
                pending_a = None
            while pending_bs and (gi, oi) >= b_release_pt(pending_bs[0][0]):
                pending_bs.pop(0)[1]()
            if gi == n_sched - 1 and oi >= 2:
                # last chunk: nothing follows to hide the tail behind, so
                # drain the score mms per-o-tile two tanhs behind (lag 2
                # keeps PE from blocking on the ACT stream)
                last_sc(oi - 2)

        def make_tails(engs=engs, nat_all=nat_all, b=b, l0=l0, ch=ch,
                       lt_n=lt_n, sc_pre=sc_pre, last_sc=last_sc):
            den4 = state[b]["den4"]
            shared = {}

            def tail_a():
                # scores: energy-stationary, v moving (out free = 1)
                if sc_pre is not None:
                    sc_ps = sc_pre
                    last_sc(OT - 2)
                    last_sc(OT - 1)
                else:
                    sc_ps = psum_sm.tile([128, lt_n], F32, name="sc_ps", tag="sm")
                    for oi in range(OT):
                        for lb in range(lt_n):
                            nc.tensor.matmul(
                                sc_ps[:, lb : lb + 1],
                                engs[oi][:, ts(lb, 128)],
                                v_bf[:, oi : oi + 1],
                                start=(oi == 0 and lb == 0),
                                stop=(oi == OT - 1 and lb == lt_n - 1),
                                skip_group_check=True,
                            )
                p_sb = small.tile([128, lt_n], BF16, name="p_sb", tag="p")
                nc.scalar.activation(p_sb, sc_ps, AF.Exp)
                nc.vector.tensor_add(den4[:, :lt_n], den4[:, :lt_n], p_sb)
                shared["p_sb"] = p_sb

            def tail_b():
                p_sb = shared["p_sb"]
                if l0 == 0:
                    state[b]["ctx"] = psum_cx.tile(
                        [128, OT], F32, name="ctx_ps", tag="cx"
                    )
                ctx_ps = state[b]["ctx"]
                # context: nat-stationary, p moving (out free = 1)
                for lt in range(lt_n):
                    for hi in range(OT):
                        nc.tensor.matmul(
                            ctx_ps[:, hi : hi + 1],
                            nat_all[:, lt, ts(hi, 128)],
                            p_sb[:, lt : lt + 1],
                            start=(l0 == 0 and lt == 0 and hi == 0),
                            stop=(l0 + ch == l_total and lt == lt_n - 1
                                  and hi == OT - 1),
                            skip_group_check=True,
                        )
                if l0 + ch == l_total:
                    # finalize batch b: context / sum(p)
                    den1 = small.tile([128, 1], F32, name="den1", tag="den1")
                    nc.vector.tensor_reduce(
                        den1, den4, mybir.AxisListType.X, mybir.AluOpType.add
                    )
                    # ones-stationary matmul partition-sums AND broadcasts the
                    # denominator to all 128 partitions in one ~free op; the
                    # unscaled copy+transpose run in parallel with it and the
                    # reciprocal is applied on the transposed 8-partition tile
                    den_rep = psum_sm.tile([128, 1], F32, name="den_rep", tag="sm")
                    nc.tensor.matmul(den_rep, ones_sq, den1, start=True, stop=True)
                    recip_bc = small.tile([128, 1], F32, name="recip_bc", tag="rbc")
                    nc.vector.reciprocal(recip_bc, den_rep)
                    ctx_sb = small.tile([128, OT], F32, name="ctx_sb", tag="ctx_sb")
                    nc.scalar.activation(ctx_sb, ctx_ps, AF.Copy, bias=0.0)
                    ctxT_ps = psum_sm.tile([OT, 128], F32, name="ctxT_ps", tag="sm")
                    nc.tensor.transpose(ctxT_ps, ctx_sb, id128)
                    out_row = small.tile([OT, 128], F32, name="out_row", tag="orow")
                    nc.vector.tensor_scalar_mul(
                        out_row, ctxT_ps, recip_bc[0:OT, :]
                    )
                    nc.sync.dma_start(out_ap[b : b + 1, :], out_row)

            return tail_a, tail_b

        if pending_a is not None:
            pending_a()
        pending_a, tb = make_tails()
        pending_bs.append((gi, tb))

    pending_a()
    for _, tb in pending_bs:
        tb()


def build_bass(b_loc=B_LOC, l_total=L, enable_asserts=False, n_repeat=1):
    """Build + schedule + compile the Bass module. Returns (nc, out_name)."""
    nc = bacc.Bacc(
        "TRN2",
        target_bir_lowering=False,
        debug=False,
        enable_asserts=enable_asserts,
        num_devices=N_CORES,
    )
    ins = {
        "hidden": nc.dram_tensor("hidden", [b_loc, H], F32, kind="ExternalInput").ap(),
        "encoder_outputs": nc.dram_tensor(
            "encoder_outputs", [l_total, b_loc, H], F32, kind="ExternalInput"
        ).ap(),
        "enc_t": nc.dram_tensor(
            "enc_t", [b_loc, H, l_total], F32, kind="ExternalInput"
        ).ap(),
        "w_dev": nc.dram_tensor(
            "w_dev", [128, H // 128, 3 * H // 128, 128], F32, kind="ExternalInput"
        ).ap(),
        "attn_b": nc.dram_tensor("attn_b", [H], F32, kind="ExternalInput").ap(),
        "v": nc.dram_tensor("v", [H], F32, kind="ExternalInput").ap(),
    }
    out = nc.dram_tensor("ctx_out", [b_loc, H], F32, kind="ExternalOutput").ap()
    with tile.TileContext(nc) as tc:
        build_attn_kernel(tc, out, ins, b_loc=b_loc, l_total=l_total,
                          n_repeat=n_repeat)
    nc.compile()
    return nc, "ctx_out"


@functools.cache
def _built():
    return build_bass()


def kernel(hidden, encoder_outputs, attn_w, attn_b, v):
    """Full-input entry point: shard over batch, run 8 cores, gather."""
    global LAST_RESULTS
    from concourse.bass_utils import run_bass_kernel_spmd

    hidden = np.ascontiguousarray(np.asarray(hidden, dtype=np.float32))
    encoder_outputs = np.ascontiguousarray(
        np.asarray(encoder_outputs, dtype=np.float32)
    )
    attn_w = np.ascontiguousarray(np.asarray(attn_w, dtype=np.float32))
    attn_b = np.ascontiguousarray(np.asarray(attn_b, dtype=np.float32))
    v = np.ascontiguousarray(np.asarray(v, dtype=np.float32))

    # Host prep: a pure-layout h-major view of the encoder tensor for the
    # transposed load, plus the blocked transposed weight
    # w_blk[p, oi, ci, o_lo] = attn_w[oi*128+o_lo, ci*128+p]. The w_e half is
    # additionally split into its e4m3 grid value and the residual (standard
    # offline fp8 weight formatting); both are shipped as fp32 and the
    # device's DMA casts finish the job (the e4m3 cast is exact by
    # construction). w_dev = [w_h | e4m3-grid(w_e) | w_e - e4m3(w_e)].
    import ml_dtypes

    enc_t_full = np.ascontiguousarray(encoder_outputs.transpose(1, 2, 0))  # [B,H,L]
    w_blk = attn_w.T.reshape(16, 128, 8, 128).transpose(1, 2, 0, 3)  # [128,8,16,128]
    w_h_blk = w_blk[:, :, :8, :]
    w_e_blk = w_blk[:, :, 8:, :]
    w_e_hi = w_e_blk.astype(ml_dtypes.float8_e4m3).astype(np.float32)
    w_e_lo = w_e_blk - w_e_hi
    w_dev = np.ascontiguousarray(
        np.concatenate([w_h_blk, w_e_hi, w_e_lo], axis=2)
    )  # [128, 8, 24, 128]

    nc, out_name = _built()
    in_maps = []
    for cidx in range(N_CORES):
        bs = slice(cidx * B_LOC, (cidx + 1) * B_LOC)
        in_maps.append(
            {
                "hidden": np.ascontiguousarray(hidden[bs]),
                "encoder_outputs": np.ascontiguousarray(encoder_outputs[:, bs, :]),
                "enc_t": np.ascontiguousarray(enc_t_full[bs]),
                "w_dev": w_dev,
                "attn_b": attn_b,
                "v": v,
            }
        )
    res = run_bass_kernel_spmd(
        nc,
        in_maps,
        core_ids=list(range(N_CORES)),
        trace=bool(os.environ.get("BASS_TRACE")),
    )
    LAST_RESULTS = res
    out = np.concatenate([res.results[cidx][out_name] for cidx in range(N_CORES)], axis=0)
    return out[None, :, :].astype(np.float32)



# revision 8
# speedup vs baseline: 1.0992x; 1.0992x over previous
"""Bahdanau (additive) attention kernel for Trainium2, 8-core data-parallel.

Math (per batch element b):
    proj[o, l]  = sum_h w_e[o, h] * enc[l, b, h]           (fp8 DoubleRow GEMM)
    energy      = tanh(proj + hidden@w_h.T + attn_b)       (bias folded into ACT)
    scores[l]   = sum_o v[o] * energy[o, l]                (energy-stationary mms)
    p           = exp(scores)                              (no max-shift needed)
    context[h]  = (sum_l p_l * enc[l, b, h]) / sum_l p_l   (nat-stationary mms)

Sharding: batch B=32 split across 8 cores (4 each); weights replicated.
No collectives.

Numerics: the main GEMM runs a SINGLE fp8e4m3 DoubleRow pass (0.5 cyc/row,
K=256/pass) — half the PE work of a two-digit scheme. The fp8 quantization
error is controlled with v-aware shaped rounding on the host: scores only
see quantization noise through sum_o v_o*tanh'*delta, so w_e is cast
column-wise with error feedback that zeroes sum_o v_o*dw[o,h] per h, and
enc is cast to zero sum_h u_h*denc[h,l] per (b,l) with u = v@w8. The
context operand (nat, fp8) is shaped to zero sum_l dnat[l,h] per (b,h).
Tails (energy, v, hidden-proj weights, p) are fp16; the context GEMM uses a
two-digit e4m3 p (p_hi + residual as one 2-column moving operand, halves
summed at finalize) so nat can be fp8 without a mixed-dtype matmul.

Per-core cost-model shape: PE ~56us (main GEMM floor 54.6), ACT ~64.5us
(tanh floor 54.6 + per-instr psum access), DMA ~20MB ~56us, so the kernel is
ACT-bound; chunks are [128,1024] psum tiles (two 4-pass DR chains, one tanh)
to amortize ACT access overhead. Scores/exp/context are pipelined into later
units' streams so no engine waits on cross-engine round trips.
"""

import functools
import os
import sys

import numpy as np

sys.path.insert(0, "/opt/trn_rl_repo")

import concourse.tile as tile  # noqa: E402
from concourse import bacc, mybir  # noqa: E402
from concourse.masks import make_identity  # noqa: E402

# This container's slim axon client lacks the NTFF profile hook module that
# run_bass_kernel_spmd's trace path imports; give it a graceful no-op fallback
# so a BASS_TRACE env var doesn't crash the run.
try:
    from antenv import axon_hooks as _axon_hooks  # noqa: F401
except Exception:
    import types as _types

    _stub = _types.ModuleType("antenv.axon_hooks")
    _stub.get_axon_ntff_profile_hook = lambda: None
    sys.modules["antenv.axon_hooks"] = _stub

import ml_dtypes  # noqa: E402

B, L, H = 32, 2048, 1024
N_CORES = 8
B_LOC = B // N_CORES
HT = H // 128  # 8 k-tiles
OT = H // 128  # 8 o-tiles

F32 = mybir.dt.float32
F16 = mybir.dt.float16
BF16 = mybir.dt.bfloat16
FP8 = mybir.dt.float8e4
AF = mybir.ActivationFunctionType
DR = mybir.MatmulPerfMode.DoubleRow
ALU = mybir.AluOpType

F8NP = ml_dtypes.float8_e4m3

LAST_RESULTS = None  # BassKernelResults of the most recent hw run (for test.py)


# ---------------------------------------------------------------------------
# Host-side shaped fp8 casts
# ---------------------------------------------------------------------------

_codes = np.arange(256, dtype=np.uint8)
_vals = _codes.view(F8NP).astype(np.float32)
E4M3_TABLE = np.unique(_vals[np.isfinite(_vals)])


def shaped_cast_e4m3(X, w, deadband=0.0):
    """Cast X[K, N] to the e4m3 grid column-wise with error feedback so that
    sum_k w[k] * (X - cast)[k, n] ~= 0 per column n. Deviates from nearest
    rounding (to the other bracketing fp8 value) only when the running
    weighted residual exceeds `deadband`. Returns float32 values on the grid.
    """
    K, N = X.shape
    X = np.ascontiguousarray(X, dtype=np.float32)
    idx = np.searchsorted(E4M3_TABLE, X)
    idx = np.clip(idx, 1, len(E4M3_TABLE) - 1)
    lo = E4M3_TABLE[idx - 1]
    hi = E4M3_TABLE[idx]
    lo = np.where(X == hi, hi, lo)

    out = np.empty((K, N), dtype=np.float32)
    r = np.zeros(N, dtype=np.float32)
    w = np.asarray(w, dtype=np.float32)
    for k in range(K):
        e_lo = X[k] - lo[k]
        e_hi = X[k] - hi[k]
        near_is_lo = (e_lo + e_hi) <= 0
        e_near = np.where(near_is_lo, e_lo, e_hi)
        e_alt = np.where(near_is_lo, e_hi, e_lo)
        y_near = np.where(near_is_lo, lo[k], hi[k])
        y_alt = np.where(near_is_lo, hi[k], lo[k])
        r_near = r + w[k] * e_near
        r_alt = r + w[k] * e_alt
        take_alt = (np.abs(r_alt) < np.abs(r_near)) & (np.abs(r_near) > deadband)
        out[k] = np.where(take_alt, y_alt, y_near)
        r = np.where(take_alt, r_alt, r_near)
    return out


def prep_host(hidden, enc, attn_w, attn_b, v, l_total=L, n_batch=B):
    """Shape-aware fp8 casts + blocked weight layouts. enc is [L, n_batch, H].

    Returns dict of full-size (unsharded) device arrays.
    """
    w_h, w_e = attn_w[:, :H], attn_w[:, H:]

    w8f = shaped_cast_e4m3(w_e, v, deadband=2e-4)  # [H(o), H(h)] on-grid
    u = v @ w8f  # [H] enc shaping weights

    enc_bhl = np.ascontiguousarray(enc.transpose(1, 2, 0))  # [nb, H, L]
    cols = np.ascontiguousarray(enc_bhl.transpose(1, 0, 2).reshape(H, n_batch * l_total))
    enc8 = (
        shaped_cast_e4m3(cols, u, deadband=5e-3)
        .reshape(H, n_batch, l_total)
        .transpose(1, 0, 2)
        .astype(F8NP)
    )  # [nb, H, L]

    # nat split: first n16 chunks of each batch's l-range ship fp16, the
    # rest fp8 with uniform shaping (zero column-sum of the quant error)
    ch = min(1024, l_total)
    n_ch = l_total // ch
    n16 = (n_ch + 1) // 2
    l16 = n16 * ch
    nat16 = np.ascontiguousarray(enc[:l16]).astype(np.float16)  # [l16, nb, H]
    if l_total > l16:
        nl = l_total - l16
        nat_cols = np.ascontiguousarray(enc[l16:].reshape(nl, n_batch * H))
        nat8 = (
            shaped_cast_e4m3(nat_cols, np.ones(nl, np.float32), deadband=0.3)
            .reshape(nl, n_batch, H)
            .astype(F8NP)
        )  # [L-l16, nb, H]
    else:
        nat8 = None

    # blocked weights: blk[p, oi, hi, o_lo] = w[oi*128+o_lo, hi*128+p]
    w8_blk = np.ascontiguousarray(
        w8f.T.reshape(HT, 128, OT, 128).transpose(1, 2, 0, 3)
    ).astype(F8NP)
    wh_blk = np.ascontiguousarray(
        w_h.T.reshape(HT, 128, OT, 128).transpose(1, 2, 0, 3)
    ).astype(np.float16)
    out = {
        "enc8": enc8,
        "nat16": nat16,
        "nat8": nat8,
        "w8": w8_blk,
        "wh16": wh_blk,
        "hidden": np.ascontiguousarray(hidden, dtype=np.float32),
        "attn_b": np.ascontiguousarray(attn_b, dtype=np.float32),
        "v": np.ascontiguousarray(v, dtype=np.float32),
    }
    if nat8 is None:
        del out["nat8"]
    return out


def _l16(l_total):
    """Per-batch l-range shipped as fp16 nat (first ceil(n_ch/2) chunks)."""
    ch = min(1024, l_total)
    n_ch = l_total // ch
    return ((n_ch + 1) // 2) * ch


# ---------------------------------------------------------------------------
# Device kernel
# ---------------------------------------------------------------------------


def build_attn_kernel(tc, out_ap, ins, b_loc=B_LOC, l_total=L, dbg=None):
    nc = tc.nc
    assert H == 1024
    dbg = dbg or {}

    from contextlib import ExitStack

    ch = min(1024, l_total)
    n_ch = l_total // ch
    LT = ch // 128          # l-blocks per chunk
    LTOT = l_total // 128   # l-blocks per batch
    units = [(b, c) for b in range(b_loc) for c in range(n_ch)]
    n_units = len(units)

    ctx = ExitStack()
    with ctx:
        const = ctx.enter_context(tc.tile_pool(name="const", bufs=1))
        enct_pool = ctx.enter_context(tc.tile_pool(name="enct", bufs=4))
        nat16_pool = ctx.enter_context(tc.tile_pool(name="nat16", bufs=3))
        nat8_pool = ctx.enter_context(tc.tile_pool(name="nat8", bufs=3))
        eng_pool = ctx.enter_context(tc.tile_pool(name="eng", bufs=12))
        small = ctx.enter_context(tc.tile_pool(name="small", bufs=10))
        psum_mm = ctx.enter_context(tc.tile_pool(name="psmm", bufs=2, space="PSUM"))
        psum_cx = ctx.enter_context(tc.tile_pool(name="pscx", bufs=1, space="PSUM"))
        psum_sm = ctx.enter_context(tc.tile_pool(name="pssm", bufs=1, space="PSUM"))
        psum_sc = ctx.enter_context(tc.tile_pool(name="pssc", bufs=1, space="PSUM"))

        n16 = (n_ch + 1) // 2   # chunks per batch with fp16 nat
        l16 = n16 * ch
        enc8 = ins["enc8"]      # [b_loc, H, l_total] fp8
        nat16d = ins["nat16"]   # [l16, b_loc, H] fp16
        nat8d = ins.get("nat8")  # [l_total-l16, b_loc, H] fp8 or absent
        w8d = ins["w8"]       # [128, OT, HT, 128] fp8
        wh16d = ins["wh16"]   # [128, OT, HT, 128] fp16

        # small HWDGE row loads first (before SWDGE claims the queue)
        attn_b_row = const.tile([1, H], F32, name="attn_b_row", tag="attn_b_row")
        nc.sync.dma_start(attn_b_row, ins["attn_b"])
        v_row = const.tile([1, H], F32, name="v_row", tag="v_row")
        nc.sync.dma_start(v_row, ins["v"])
        hid_sb = const.tile([b_loc, H], F32, name="hid_sb", tag="hid_sb")
        nc.sync.dma_start(hid_sb, ins["hidden"])

        # ---------------- chunk loads ----------------
        enct_cache = {}
        nat_cache = {}

        def load_enct(uu):
            b, c = units[uu]
            l0 = c * ch
            t = enct_pool.tile([128, HT, ch], FP8, name="enct8", tag="enct")
            nc.gpsimd.dma_start(
                t, enc8[b, :, l0 : l0 + ch].rearrange("(hi p) l -> p hi l", p=128)
            )
            enct_cache[uu] = t

        def load_nat(uu):
            b, c = units[uu]
            l0 = c * ch
            if c < n16:
                t = nat16_pool.tile([128, LT, H], F16, name="nat16", tag="nat16")
                srcv = nat16d[l0 : l0 + ch, b, :]
            else:
                t = nat8_pool.tile([128, LT, H], FP8, name="nat8", tag="nat8")
                srcv = nat8d[l0 - l16 : l0 - l16 + ch, b, :]
            nc.gpsimd.dma_start(t, srcv.rearrange("(lt p) h -> p lt h", p=128))
            nat_cache[uu] = t

        # ---------------- weights (per-oi, just-in-time order) -------------
        w8_all = const.tile([128, OT, HT, 128], FP8, name="w8_all", tag="w8_all")
        wh_all = const.tile([128, OT, HT, 128], F16, name="wh_all", tag="wh_all")

        def load_w8(oi):
            nc.gpsimd.dma_start(w8_all[:, oi], w8d[:, oi])

        def load_wh(oi):
            nc.gpsimd.dma_start(wh_all[:, oi], wh16d[:, oi])

        load_w8(0)
        load_enct(0)
        load_wh(0)
        for oi in range(1, 4):
            load_w8(oi)
            load_wh(oi)
        if n_units > 1:
            load_enct(1)
        for oi in range(4, OT):
            load_w8(oi)
            load_wh(oi)
        load_nat(0)
        if n_units > 2:
            load_enct(2)
        if n_units > 1:
            load_nat(1)
        if n_units > 3:
            load_enct(3)

        # identities / constants (gpsimd iota work, overlaps DMA waits)
        warm_sb = const.tile([128, 128], BF16, name="warm_sb", tag="warm_sb")
        nc.gpsimd.memset(warm_sb, 0.0)
        idb = const.tile([b_loc, b_loc], F32, name="idb", tag="idb")
        make_identity(nc, idb)
        id1 = const.tile([1, 1], F32, name="id1", tag="id1")
        make_identity(nc, id1)
        id128 = const.tile([128, 128], F32, name="id128", tag="id128")
        make_identity(nc, id128)
        ones_sq = const.tile([128, 128], F32, name="ones_sq", tag="ones_sq")
        nc.gpsimd.memset(ones_sq, 1.0)

        # PE warm-up ramps the tensor engine out of low p-states
        warm_ps = psum_sm.tile([128, 128], F32, name="warm_ps", tag="sm")
        for _ in range(14):
            nc.tensor.matmul(warm_ps, warm_sb, warm_sb, start=True, stop=True,
                             skip_group_check=True)

        # hidden/attn_b/v transposes batched into psum columns
        hT = const.tile([128, HT * b_loc], F16, name="hT", tag="hT")
        ps_hT = psum_sm.tile([128, HT * b_loc], F32, name="ps_hT", tag="sm")
        for hi in range(HT):
            nc.tensor.matmul(
                ps_hT[:, hi * b_loc : (hi + 1) * b_loc],
                hid_sb[:, hi * 128 : (hi + 1) * 128],
                idb,
                is_transpose=True,
                start=(hi == 0),
                stop=True,
                skip_group_check=True,
            )
        nc.vector.tensor_copy(hT, ps_hT)
        attn_b_sb = const.tile([128, OT], F32, name="attn_b_sb", tag="attn_b_sb")
        v16 = const.tile([128, OT], F16, name="v16", tag="v16")
        ps_bv = psum_sm.tile([128, 2 * OT], F32, name="ps_bv", tag="sm")
        for oi in range(OT):
            nc.tensor.matmul(
                ps_bv[:, oi : oi + 1],
                attn_b_row[:, oi * 128 : (oi + 1) * 128],
                id1,
                is_transpose=True,
                start=(oi == 0),
                stop=True,
                skip_group_check=True,
            )
            nc.tensor.matmul(
                ps_bv[:, OT + oi : OT + oi + 1],
                v_row[:, oi * 128 : (oi + 1) * 128],
                id1,
                is_transpose=True,
                start=False,
                stop=True,
                skip_group_check=True,
            )
        nc.vector.tensor_copy(attn_b_sb, ps_bv[:, :OT])
        nc.vector.tensor_copy(v16, ps_bv[:, OT:])

        # bias_sb[:, oi*b_loc + b] = (hidden @ w_h.T)[b, oi-tile] + attn_b
        bias_sb = const.tile([128, OT * b_loc], F32, name="bias_sb", tag="bias_sb")

        def emit_bias(oi):
            hp_ps = psum_cx.tile([128, b_loc], F32, name="hp_ps", tag="cx")
            for hi in range(HT):
                nc.tensor.matmul(
                    hp_ps,
                    wh_all[:, oi, hi, :],
                    hT[:, hi * b_loc : (hi + 1) * b_loc],
                    start=(hi == 0),
                    stop=(hi == HT - 1),
                )
            nc.vector.tensor_scalar_add(
                bias_sb[:, oi * b_loc : (oi + 1) * b_loc],
                hp_ps,
                attn_b_sb[:, oi : oi + 1],
            )

        # ---------------- pipelined main loop ----------------
        sc_ps = {}      # b -> psum [128, LTOT]
        p16s = {}       # b -> sbuf fp16 [128, LTOT]
        p8s = {}        # b -> sbuf fp8 [128, LTOT, 2]
        unit_nat = {}   # u -> nat tile
        hooks = {}      # (u, oi) -> [callback]

        def emit_score(b, c, oi, engs):
            # start exactly once per psum bank (it resets the whole bank)
            for lb in range(LT):
                nc.tensor.matmul(
                    sc_ps[b][:, c * LT + lb : c * LT + lb + 1],
                    engs[oi][:, lb * 128 : (lb + 1) * 128],
                    v16[:, oi : oi + 1],
                    start=(c == 0 and oi == 0 and lb == 0),
                    stop=(c == n_ch - 1 and oi == OT - 1 and lb == LT - 1),
                    skip_group_check=True,
                )

        def emit_exp_psplit(b):
            p16 = small.tile([128, LTOT], F16, name="p16", tag="p16")
            nc.scalar.activation(p16, sc_ps[b], AF.Exp)
            p16s[b] = p16
            if n_ch > n16:
                # two-digit e4m3 p for the fp8-nat context matmuls
                p8 = small.tile([128, LTOT, 2], FP8, name="p8", tag="p8")
                nc.vector.tensor_copy(p8[:, :, 0], p16)
                nc.vector.tensor_sub(p8[:, :, 1], p16, p8[:, :, 0])
                p8s[b] = p8
            if b == 0 and "p16" in dbg:
                nc.sync.dma_start(dbg["p16"], p16)
                nc.sync.dma_start(dbg["p8"], p8)
                sc_sb = small.tile([128, LTOT], F32, name="sc_sb", tag="dbg")
                nc.vector.tensor_copy(sc_sb, sc_ps[b])
                nc.sync.dma_start(dbg["sc"], sc_sb)

        def emit_ctx_finalize(b):
            p16 = p16s[b]
            p8 = p8s.get(b)
            ctx_ps = psum_cx.tile([128, OT], F32, name="ctx_ps", tag="cx")
            first = [True]
            for c2 in range(n_ch):
                natt = unit_nat.pop((b, c2))
                for lt in range(LT):
                    for hi in range(OT):
                        last = (c2 == n_ch - 1 and lt == LT - 1
                                and hi == OT - 1)
                        if c2 < n16:
                            movs = [p16[:, c2 * LT + lt : c2 * LT + lt + 1]]
                        else:
                            movs = [p8[:, c2 * LT + lt, 0:1],
                                    p8[:, c2 * LT + lt, 1:2]]
                        for di, mov in enumerate(movs):
                            nc.tensor.matmul(
                                ctx_ps[:, hi : hi + 1],
                                natt[:, lt, hi * 128 : (hi + 1) * 128],
                                mov,
                                start=first[0],
                                stop=(last and di == len(movs) - 1),
                                skip_group_check=True,
                            )
                            first[0] = False
            if b == 0 and "ctx" in dbg:
                cx_sb = small.tile([128, 2 * OT], F32, name="cx_sb", tag="dbg")
                nc.vector.tensor_copy(cx_sb, ctx_ps)
                nc.sync.dma_start(dbg["ctx"], cx_sb)
            den1 = small.tile([128, 1], F32, name="den1", tag="den1")
            nc.vector.tensor_reduce(
                den1, p16, mybir.AxisListType.X, mybir.AluOpType.add
            )
            # ones-stationary matmul partition-sums AND broadcasts the
            # denominator to all 128 partitions in one ~free op
            den_rep = psum_sm.tile([128, 1], F32, name="den_rep", tag="sm")
            nc.tensor.matmul(den_rep, ones_sq, den1, start=True, stop=True)
            recip_bc = small.tile([128, 1], F32, name="recip_bc", tag="rbc")
            nc.vector.reciprocal(recip_bc, den_rep)
            ctx_sb = small.tile([128, OT], F32, name="ctx_sb", tag="ctx_sb")
            nc.vector.tensor_copy(ctx_sb, ctx_ps)
            ctxT_ps = psum_sm.tile([OT, 128], F32, name="ctxT_ps", tag="sm")
            nc.tensor.transpose(ctxT_ps, ctx_sb, id128)
            out_row = small.tile([OT, 128], F32, name="out_row", tag="orow")
            nc.vector.tensor_scalar_mul(out_row, ctxT_ps, recip_bc[0:OT, :])
            nc.sync.dma_start(out_ap[b : b + 1, :], out_row)

        for u, (b, c) in enumerate(units):
            if u not in enct_cache:
                load_enct(u)
            enct8 = enct_cache.pop(u)
            if u not in nat_cache:
                load_nat(u)
            unit_nat[(b, c)] = nat_cache.pop(u)
            if u + 3 < n_units and u + 3 not in enct_cache:
                load_enct(u + 3)
            if u + 1 < n_units and u + 1 not in nat_cache:
                load_nat(u + 1)
            if c == 0:
                sc_ps[b] = psum_sc.tile([128, LTOT], F32, name="sc_ps", tag="sc")

            engs = [None] * OT
            for oi in range(OT):
                mm_ps = psum_mm.tile([128, ch], F32, name="mm_ps", tag="mm")
                for half in range(ch // 512):
                    hs = slice(half * 512, (half + 1) * 512)
                    for q in range(HT // 2):
                        nc.tensor.matmul(
                            mm_ps[:, hs],
                            w8_all[:, oi, 2 * q : 2 * q + 2, :],
                            enct8[:, 2 * q : 2 * q + 2, hs],
                            start=(q == 0),
                            stop=(q == HT // 2 - 1),
                            perf_mode=DR,
                        )
                if u == 0:
                    emit_bias(oi)
                eng = eng_pool.tile([128, ch], F16, name="eng", tag="eng")
                nc.scalar.activation(
                    eng,
                    mm_ps,
                    AF.Tanh,
                    bias=bias_sb[:, oi * b_loc + b : oi * b_loc + b + 1],
                    scale=1.0,
                )
                engs[oi] = eng
                if u == 0 and oi == 0 and "eng00" in dbg:
                    nc.sync.dma_start(dbg["eng00"], eng)
                if u == 0 and oi == OT - 1 and "bias" in dbg:
                    nc.sync.dma_start(dbg["bias"], bias_sb)
                for cb in hooks.pop((u, oi), ()):
                    cb()
                if oi >= 2:
                    emit_score(b, c, oi - 2, engs)

            # defer this unit's last two score columns + tails into the next
            # unit's stream so PE never blocks on the ACT queue
            def _mk(fn, *args):
                return lambda: fn(*args)

            if u + 1 < n_units:
                hooks.setdefault((u + 1, 0), []).append(
                    _mk(emit_score, b, c, OT - 2, engs)
                )
                nxt1 = hooks.setdefault((u + 1, 1), [])
                nxt1.append(_mk(emit_score, b, c, OT - 1, engs))
                if c == n_ch - 1:
                    nxt1.append(_mk(emit_exp_psplit, b))
                    hooks.setdefault((u + 1, 4), []).append(
                        _mk(emit_ctx_finalize, b)
                    )
            else:
                emit_score(b, c, OT - 2, engs)
                emit_score(b, c, OT - 1, engs)
                emit_exp_psplit(b)
                emit_ctx_finalize(b)


def build_bass(b_loc=B_LOC, l_total=L, enable_asserts=False, debug_taps=False):
    """Build + schedule + compile the Bass module. Returns (nc, out_name)."""
    nc = bacc.Bacc(
        "TRN2",
        target_bir_lowering=False,
        debug=False,
        enable_asserts=enable_asserts,
        num_devices=N_CORES,
    )
    ins = {
        "hidden": nc.dram_tensor("hidden", [b_loc, H], F32, kind="ExternalInput").ap(),
        "enc8": nc.dram_tensor(
            "enc8", [b_loc, H, l_total], FP8, kind="ExternalInput"
        ).ap(),
        "nat16": nc.dram_tensor(
            "nat16", [_l16(l_total), b_loc, H], F16, kind="ExternalInput"
        ).ap(),
        "w8": nc.dram_tensor(
            "w8", [128, OT, HT, 128], FP8, kind="ExternalInput"
        ).ap(),
        "wh16": nc.dram_tensor(
            "wh16", [128, OT, HT, 128], F16, kind="ExternalInput"
        ).ap(),
        "attn_b": nc.dram_tensor("attn_b", [H], F32, kind="ExternalInput").ap(),
        "v": nc.dram_tensor("v", [H], F32, kind="ExternalInput").ap(),
    }
    if l_total > _l16(l_total):
        ins["nat8"] = nc.dram_tensor(
            "nat8", [l_total - _l16(l_total), b_loc, H], FP8, kind="ExternalInput"
        ).ap()
    out = nc.dram_tensor("ctx_out", [b_loc, H], F32, kind="ExternalOutput").ap()
    dbg = {}
    if debug_taps:
        ch = min(1024, l_total)
        LTOT = l_total // 128
        dbg = {
            "eng00": nc.dram_tensor("dbg_eng00", [128, ch], F16, kind="ExternalOutput").ap(),
            "bias": nc.dram_tensor("dbg_bias", [128, OT * b_loc], F32, kind="ExternalOutput").ap(),
            "p16": nc.dram_tensor("dbg_p16", [128, LTOT], F16, kind="ExternalOutput").ap(),
            "p8": nc.dram_tensor("dbg_p8", [128, LTOT, 2], FP8, kind="ExternalOutput").ap(),
            "sc": nc.dram_tensor("dbg_sc", [128, LTOT], F32, kind="ExternalOutput").ap(),
            "ctx": nc.dram_tensor("dbg_ctx", [128, OT], F32, kind="ExternalOutput").ap(),
        }
    with tile.TileContext(nc) as tc:
        build_attn_kernel(tc, out, ins, b_loc=b_loc, l_total=l_total, dbg=dbg)
    nc.compile()
    return nc, "ctx_out"


@functools.cache
def _built():
    return build_bass()


def kernel(hidden, encoder_outputs, attn_w, attn_b, v):
    """Full-input entry point: shard over batch, run 8 cores, gather."""
    global LAST_RESULTS
    from concourse.bass_utils import run_bass_kernel_spmd

    hidden = np.ascontiguousarray(np.asarray(hidden, dtype=np.float32))
    encoder_outputs = np.ascontiguousarray(
        np.asarray(encoder_outputs, dtype=np.float32)
    )
    attn_w = np.ascontiguousarray(np.asarray(attn_w, dtype=np.float32))
    attn_b = np.ascontiguousarray(np.asarray(attn_b, dtype=np.float32))
    v = np.ascontiguousarray(np.asarray(v, dtype=np.float32))

    prep = prep_host(hidden, encoder_outputs, attn_w, attn_b, v)

    nc, out_name = _built()
    in_maps = []
    for cidx in range(N_CORES):
        bs = slice(cidx * B_LOC, (cidx + 1) * B_LOC)
        in_maps.append(
            {
                "hidden": prep["hidden"][bs],
                "enc8": np.ascontiguousarray(prep["enc8"][bs]),
                "nat16": np.ascontiguousarray(prep["nat16"][:, bs, :]),
                **({"nat8": np.ascontiguousarray(prep["nat8"][:, bs, :])}
                   if "nat8" in prep else {}),
                "w8": prep["w8"],
                "wh16": prep["wh16"],
                "attn_b": prep["attn_b"],
                "v": prep["v"],
            }
        )
    res = run_bass_kernel_spmd(
        nc,
        in_maps,
        core_ids=list(range(N_CORES)),
        trace=bool(os.environ.get("BASS_TRACE")),
    )
    LAST_RESULTS = res
    out = np.concatenate(
        [res.results[cidx][out_name] for cidx in range(N_CORES)], axis=0
    )
    return out[None, :, :].astype(np.float32)


# revision 24
# speedup vs baseline: 1.3239x; 1.2044x over previous
"""Bahdanau (additive) attention kernel for Trainium2, 8-core data-parallel.

Math (per batch element b):
    proj[o, l]  = sum_h w_e[o, h] * enc[l, b, h]           (fp8 DoubleRow GEMM)
    energy      = tanh(proj + hidden@w_h.T + attn_b)       (bias folded into ACT)
    scores[l]   = sum_o v[o] * energy[o, l]                (energy-stationary mms)
    p           = exp(scores)                              (no max-shift needed)
    context[h]  = (sum_l p_l * enc[l, b, h]) / sum_l p_l   (nat-stationary mms)

Sharding: batch B=32 split across 8 cores (4 each); weights replicated.
No collectives.

Numerics: the main GEMM runs a SINGLE fp8e4m3 DoubleRow pass (0.5 cyc/row,
K=256/pass) — half the PE work of a two-digit scheme. The fp8 quantization
error is controlled with v-aware shaped rounding on the host: scores only
see quantization noise through sum_o v_o*tanh'*delta, so w_e is cast
column-wise with error feedback that zeroes sum_o v_o*dw[o,h] per h, and
enc is cast to zero sum_h u_h*denc[h,l] per (b,l) with u = v@w8. The
context operand (nat, fp8) is shaped to zero sum_l dnat[l,h] per (b,h).
Tails (energy, v, hidden-proj weights, p) are fp16; the context GEMM uses a
two-digit e4m3 p (p_hi + residual as one 2-column moving operand, halves
summed at finalize) so nat can be fp8 without a mixed-dtype matmul.

Per-core cost-model shape: PE ~56us (main GEMM floor 54.6), ACT ~64.5us
(tanh floor 54.6 + per-instr psum access), DMA ~20MB ~56us, so the kernel is
ACT-bound; chunks are [128,1024] psum tiles (two 4-pass DR chains, one tanh)
to amortize ACT access overhead. Scores/exp/context are pipelined into later
units' streams so no engine waits on cross-engine round trips.
"""

import functools
import os
import sys

import numpy as np

sys.path.insert(0, "/opt/trn_rl_repo")

import concourse.tile as tile  # noqa: E402
from concourse import bacc, mybir  # noqa: E402
from concourse.masks import make_identity  # noqa: E402

# This container's slim axon client lacks the NTFF profile hook module that
# run_bass_kernel_spmd's trace path imports; give it a graceful no-op fallback
# so a BASS_TRACE env var doesn't crash the run.
try:
    from antenv import axon_hooks as _axon_hooks  # noqa: F401
except Exception:
    import types as _types

    _stub = _types.ModuleType("antenv.axon_hooks")
    _stub.get_axon_ntff_profile_hook = lambda: None
    sys.modules["antenv.axon_hooks"] = _stub

import ml_dtypes  # noqa: E402

B, L, H = 32, 2048, 1024
N_CORES = 8
B_LOC = B // N_CORES
HT = H // 128  # 8 k-tiles
OT = H // 128  # 8 o-tiles

F32 = mybir.dt.float32
F16 = mybir.dt.float16
BF16 = mybir.dt.bfloat16
FP8 = mybir.dt.float8e4
AF = mybir.ActivationFunctionType
DR = mybir.MatmulPerfMode.DoubleRow
ALU = mybir.AluOpType

F8NP = ml_dtypes.float8_e4m3

LAST_RESULTS = None  # BassKernelResults of the most recent hw run (for test.py)

# schedule-tuning knobs (read at build time; see tune_sweep.py)
TUNE = {
    "w_pairs_before_e1": 3,  # weight pairs issued before the enct(1) load
    "startup_tail": "e2n0n1e3",  # order of enct(2),enct(3),nat(0),nat(1)
    "enct_ahead": 3,          # in-loop enct prefetch distance
    "nat_ahead": 1,           # in-loop nat prefetch distance
    "ctx_hook": 4,            # o-tile slot of the ctx hook in the next unit
    "exp_hook": 1,            # o-tile slot of the score7+exp hook
    "warm": 40,               # PE warm-up matmuls
}


# ---------------------------------------------------------------------------
# Host-side shaped fp8 casts
# ---------------------------------------------------------------------------

_codes = np.arange(256, dtype=np.uint8)
_vals = _codes.view(F8NP).astype(np.float32)
E4M3_TABLE = np.unique(_vals[np.isfinite(_vals)])


def shaped_cast_e4m3(X, w, deadband=0.0):
    """Cast X[K, N] to the e4m3 grid column-wise with error feedback so that
    sum_k w[k] * (X - cast)[k, n] ~= 0 per column n. Deviates from nearest
    rounding (to the other bracketing fp8 value) only when the running
    weighted residual exceeds `deadband`. Returns float32 values on the grid.
    """
    K, N = X.shape
    X = np.ascontiguousarray(X, dtype=np.float32)
    idx = np.searchsorted(E4M3_TABLE, X)
    idx = np.clip(idx, 1, len(E4M3_TABLE) - 1)
    lo = E4M3_TABLE[idx - 1]
    hi = E4M3_TABLE[idx]
    lo = np.where(X == hi, hi, lo)

    out = np.empty((K, N), dtype=np.float32)
    r = np.zeros(N, dtype=np.float32)
    w = np.asarray(w, dtype=np.float32)
    for k in range(K):
        e_lo = X[k] - lo[k]
        e_hi = X[k] - hi[k]
        near_is_lo = (e_lo + e_hi) <= 0
        e_near = np.where(near_is_lo, e_lo, e_hi)
        e_alt = np.where(near_is_lo, e_hi, e_lo)
        y_near = np.where(near_is_lo, lo[k], hi[k])
        y_alt = np.where(near_is_lo, hi[k], lo[k])
        r_near = r + w[k] * e_near
        r_alt = r + w[k] * e_alt
        take_alt = (np.abs(r_alt) < np.abs(r_near)) & (np.abs(r_near) > deadband)
        out[k] = np.where(take_alt, y_alt, y_near)
        r = np.where(take_alt, r_alt, r_near)
    return out


def prep_host(hidden, enc, attn_w, attn_b, v, l_total=L, n_batch=B):
    """Shape-aware fp8 casts + blocked weight layouts. enc is [L, n_batch, H].

    Returns dict of full-size (unsharded) device arrays.
    """
    w_h, w_e = attn_w[:, :H], attn_w[:, H:]

    w8f = shaped_cast_e4m3(w_e, v, deadband=2e-4)  # [H(o), H(h)] on-grid
    u = v @ w8f  # [H] enc shaping weights

    enc_bhl = np.ascontiguousarray(enc.transpose(1, 2, 0))  # [nb, H, L]
    cols = np.ascontiguousarray(enc_bhl.transpose(1, 0, 2).reshape(H, n_batch * l_total))
    enc8 = (
        shaped_cast_e4m3(cols, u, deadband=5e-3)
        .reshape(H, n_batch, l_total)
        .transpose(1, 0, 2)
        .astype(F8NP)
    )  # [nb, H, L]

    # nat split: first n16 chunks of each batch's l-range ship fp16, the
    # rest fp8 with uniform shaping (zero column-sum of the quant error)
    ch = min(1024, l_total)
    n_ch = l_total // ch
    n16 = (n_ch + 1) // 2
    l16 = n16 * ch
    nat16 = np.ascontiguousarray(enc[:l16]).astype(np.float16)  # [l16, nb, H]
    if l_total > l16:
        nl = l_total - l16
        nat_cols = np.ascontiguousarray(enc[l16:].reshape(nl, n_batch * H))
        nat8 = (
            shaped_cast_e4m3(nat_cols, np.ones(nl, np.float32), deadband=0.3)
            .reshape(nl, n_batch, H)
            .astype(F8NP)
        )  # [L-l16, nb, H]
    else:
        nat8 = None

    # blocked weights: blk[p, oi, hi, o_lo] = w[oi*128+o_lo, hi*128+p]
    w8_blk = np.ascontiguousarray(
        w8f.T.reshape(HT, 128, OT, 128).transpose(1, 2, 0, 3)
    ).astype(F8NP)
    wh_blk = np.ascontiguousarray(
        w_h.T.reshape(HT, 128, OT, 128).transpose(1, 2, 0, 3)
    ).astype(np.float16)
    out = {
        "enc8": enc8,
        "nat16": nat16,
        "nat8": nat8,
        "w8": w8_blk,
        "wh16": wh_blk,
        "hidden": np.ascontiguousarray(hidden, dtype=np.float32),
        "attn_b": np.ascontiguousarray(attn_b, dtype=np.float32),
        "v": np.ascontiguousarray(v, dtype=np.float32),
    }
    if nat8 is None:
        del out["nat8"]
    return out


def _l16(l_total):
    """Per-batch l-range shipped as fp16 nat (first ceil(n_ch/2) chunks)."""
    ch = min(1024, l_total)
    n_ch = l_total // ch
    return ((n_ch + 1) // 2) * ch


# ---------------------------------------------------------------------------
# Device kernel
# ---------------------------------------------------------------------------


def build_attn_kernel(tc, out_ap, ins, b_loc=B_LOC, l_total=L, dbg=None):
    nc = tc.nc
    assert H == 1024
    dbg = dbg or {}

    from contextlib import ExitStack

    ch = min(1024, l_total)
    n_ch = l_total // ch
    LT = ch // 128          # l-blocks per chunk
    LTOT = l_total // 128   # l-blocks per batch
    units = [(b, c) for b in range(b_loc) for c in range(n_ch)]
    if n_ch > 1:
        # process the last batch's chunks in reverse so the final exposed
        # tail is the fp16-nat chunk (no p-split on the critical path)
        units[-n_ch:] = units[-n_ch:][::-1]
    n_units = len(units)
    # chronological bookkeeping per batch (chunk processing order may vary)
    chunks_done = {b: 0 for b in range(b_loc)}

    ctx = ExitStack()
    with ctx:
        const = ctx.enter_context(tc.tile_pool(name="const", bufs=1))
        enct_pool = ctx.enter_context(tc.tile_pool(name="enct", bufs=4))
        nat16_pool = ctx.enter_context(tc.tile_pool(name="nat16", bufs=3))
        nat8_pool = ctx.enter_context(tc.tile_pool(name="nat8", bufs=3))
        eng_pool = ctx.enter_context(tc.tile_pool(name="eng", bufs=12))
        small = ctx.enter_context(tc.tile_pool(name="small", bufs=10))
        psum_mm = ctx.enter_context(tc.tile_pool(name="psmm", bufs=2, space="PSUM"))
        psum_cx = ctx.enter_context(tc.tile_pool(name="pscx", bufs=1, space="PSUM"))
        psum_sm = ctx.enter_context(tc.tile_pool(name="pssm", bufs=1, space="PSUM"))
        psum_sc = ctx.enter_context(tc.tile_pool(name="pssc", bufs=2, space="PSUM"))

        n16 = (n_ch + 1) // 2   # chunks per batch with fp16 nat
        l16 = n16 * ch
        enc8 = ins["enc8"]      # [b_loc, H, l_total] fp8
        nat16d = ins["nat16"]   # [l16, b_loc, H] fp16
        nat8d = ins.get("nat8")  # [l_total-l16, b_loc, H] fp8 or absent
        w8d = ins["w8"]       # [128, OT, HT, 128] fp8
        wh16d = ins["wh16"]   # [128, OT, HT, 128] fp16

        attn_b_row = const.tile([1, H], F32, name="attn_b_row", tag="attn_b_row")
        v_row = const.tile([1, H], F32, name="v_row", tag="v_row")
        hid_sb = const.tile([b_loc, H], F32, name="hid_sb", tag="hid_sb")

        # ---------------- chunk loads ----------------
        enct_cache = {}
        nat_cache = {}

        def load_enct(uu, split=False):
            b, c = units[uu]
            l0 = c * ch
            t = enct_pool.tile([128, HT, ch], FP8, name="enct8", tag="enct")
            if split:
                # two half-l transfers: unit 0's first GEMM half can start as
                # soon as the first 512 l-columns land
                for hs in (slice(0, ch // 2), slice(ch // 2, ch)):
                    nc.gpsimd.dma_start(
                        t[:, :, hs],
                        enc8[b, :, l0 + hs.start : l0 + hs.stop].rearrange(
                            "(hi p) l -> p hi l", p=128
                        ),
                    )
            else:
                nc.gpsimd.dma_start(
                    t, enc8[b, :, l0 : l0 + ch].rearrange("(hi p) l -> p hi l", p=128)
                )
            enct_cache[uu] = t

        def load_nat(uu):
            b, c = units[uu]
            l0 = c * ch
            if c < n16:
                t = nat16_pool.tile([128, LT, H], F16, name="nat16", tag="nat16")
                srcv = nat16d[l0 : l0 + ch, b, :]
            else:
                t = nat8_pool.tile([128, LT, H], FP8, name="nat8", tag="nat8")
                srcv = nat8d[l0 - l16 : l0 - l16 + ch, b, :]
            nc.gpsimd.dma_start(t, srcv.rearrange("(lt p) h -> p lt h", p=128))
            nat_cache[uu] = t

        # ---------------- weights (per-oi, just-in-time order) -------------
        w8_all = const.tile([128, OT, HT, 128], FP8, name="w8_all", tag="w8_all")
        wh_all = const.tile([128, OT, HT, 128], F16, name="wh_all", tag="wh_all")

        def load_w8(oi):
            nc.gpsimd.dma_start(w8_all[:, oi], w8d[:, oi])

        def load_wh(oi):
            nc.gpsimd.dma_start(wh_all[:, oi], wh16d[:, oi])

        # first o-tile's weights ride the (otherwise idle) HWDGE path so
        # they land while the Pool queue is still issuing enct(0); hidden
        # first — the hT transposes feed the bias chain, which gates the
        # first tanh
        nc.sync.dma_start(hid_sb, ins["hidden"])
        nc.sync.dma_start(w8_all[:, 0], w8d[:, 0])
        nc.sync.dma_start(wh_all[:, 0], wh16d[:, 0])
        nc.sync.dma_start(attn_b_row, ins["attn_b"])
        nc.sync.dma_start(v_row, ins["v"])

        # Pool-queue order is the startup critical path: the first GEMM needs
        # enct(0) (2.9us transfer) so its SWDGE issue goes absolutely first;
        # the memsets/iotas (needed by the PE warm-up and transposes) come
        # next, ahead of the remaining ~1us-each DMA issues.
        load_enct(0)

        warm_sb = const.tile([128, 128], BF16, name="warm_sb", tag="warm_sb")
        nc.gpsimd.memset(warm_sb, 0.0)
        idb = const.tile([b_loc, b_loc], F32, name="idb", tag="idb")
        make_identity(nc, idb)
        id1 = const.tile([1, 1], F32, name="id1", tag="id1")
        make_identity(nc, id1)
        id128 = const.tile([128, 128], F32, name="id128", tag="id128")
        make_identity(nc, id128)
        ones_sq = const.tile([128, 128], F32, name="ones_sq", tag="ones_sq")
        nc.gpsimd.memset(ones_sq, 1.0)

        npre = TUNE["w_pairs_before_e1"]
        for oi in range(1, 1 + npre):
            load_w8(oi)
            load_wh(oi)
        if n_units > 1:
            load_enct(1)
        for oi in range(1 + npre, OT):
            load_w8(oi)
            load_wh(oi)
        for tok in TUNE["startup_tail"].replace("e", " e").replace("n", " n").split():
            kind, idx = tok[0], int(tok[1])
            if kind == "e" and n_units > idx and idx not in enct_cache:
                load_enct(idx)
            elif kind == "n" and n_units > idx and idx not in nat_cache:
                load_nat(idx)

        # dummy tanh: forces the ACT function-table load to happen during
        # the startup idle window rather than right before the first real tanh
        act_warm = const.tile([1, 1], F32, name="act_warm", tag="act_warm")
        nc.scalar.activation(act_warm, warm_sb[0:1, 0:1], AF.Tanh)

        # PE warm-up ramps the tensor engine out of low p-states; enough of
        # them to stay busy until the first real GEMM chain (~5.5us) so the
        # p-state doesn't decay back to mid speed
        warm_ps = psum_sm.tile([128, 128], F32, name="warm_ps", tag="sm")
        for _ in range(TUNE["warm"]):
            nc.tensor.matmul(warm_ps, warm_sb, warm_sb, start=True, stop=True,
                             skip_group_check=True)

        # hidden/attn_b/v transposes batched into psum columns
        hT = const.tile([128, HT * b_loc], F16, name="hT", tag="hT")
        ps_hT = psum_sm.tile([128, HT * b_loc], F32, name="ps_hT", tag="sm")
        for hi in range(HT):
            nc.tensor.matmul(
                ps_hT[:, hi * b_loc : (hi + 1) * b_loc],
                hid_sb[:, hi * 128 : (hi + 1) * 128],
                idb,
                is_transpose=True,
                start=(hi == 0),
                stop=True,
                skip_group_check=True,
            )
        nc.vector.tensor_copy(hT, ps_hT)
        attn_b_sb = const.tile([128, OT], F32, name="attn_b_sb", tag="attn_b_sb")
        v16 = const.tile([128, OT], F16, name="v16", tag="v16")
        ps_bv = psum_sm.tile([128, 2 * OT], F32, name="ps_bv", tag="sm")
        for oi in range(OT):
            nc.tensor.matmul(
                ps_bv[:, oi : oi + 1],
                attn_b_row[:, oi * 128 : (oi + 1) * 128],
                id1,
                is_transpose=True,
                start=(oi == 0),
                stop=True,
                skip_group_check=True,
            )
            nc.tensor.matmul(
                ps_bv[:, OT + oi : OT + oi + 1],
                v_row[:, oi * 128 : (oi + 1) * 128],
                id1,
                is_transpose=True,
                start=False,
                stop=True,
                skip_group_check=True,
            )
        nc.vector.tensor_copy(attn_b_sb, ps_bv[:, :OT])
        nc.vector.tensor_copy(v16, ps_bv[:, OT:])

        # bias_sb[:, oi*b_loc + b] = (hidden @ w_h.T)[b, oi-tile] + attn_b
        bias_sb = const.tile([128, OT * b_loc], F32, name="bias_sb", tag="bias_sb")

        def emit_bias(oi):
            hp_ps = psum_cx.tile([128, b_loc], F32, name="hp_ps", tag="cx")
            for hi in range(HT):
                nc.tensor.matmul(
                    hp_ps,
                    wh_all[:, oi, hi, :],
                    hT[:, hi * b_loc : (hi + 1) * b_loc],
                    start=(hi == 0),
                    stop=(hi == HT - 1),
                )
            nc.vector.tensor_scalar_add(
                bias_sb[:, oi * b_loc : (oi + 1) * b_loc],
                hp_ps,
                attn_b_sb[:, oi : oi + 1],
            )

        emit_bias(0)

        # ---------------- pipelined main loop ----------------
        sc_ps = {}      # b -> psum [128, LTOT]
        p16s = {}       # b -> sbuf fp16 [128, LTOT]
        p8s = {}        # b -> sbuf fp8 [128, LTOT, 2]
        unit_nat = {}   # u -> nat tile
        hooks = {}      # (u, oi) -> [callback]

        sc_started = set()
        sc_n = {b: 0 for b in range(b_loc)}

        def emit_score(b, c, oi, engs):
            # start exactly once per psum bank (it resets the whole bank)
            for lb in range(LT):
                sc_n[b] += 1
                nc.tensor.matmul(
                    sc_ps[b][:, c * LT + lb : c * LT + lb + 1],
                    engs[oi][:, lb * 128 : (lb + 1) * 128],
                    v16[:, oi : oi + 1],
                    start=(b not in sc_started and not sc_started.add(b)),
                    stop=(sc_n[b] == n_ch * OT * LT),
                    skip_group_check=True,
                )

        def emit_exp_psplit(b, c):
            if b not in p16s:
                p16s[b] = small.tile([128, LTOT], F16, name="p16", tag="p16")
                if n_ch > n16:
                    p8s[b] = small.tile([128, LTOT, 2], FP8, name="p8", tag="p8")
            p16 = p16s[b]
            sl = slice(c * LT, (c + 1) * LT)
            nc.scalar.activation(p16[:, sl], sc_ps[b][:, sl], AF.Exp)
            if c >= n16:
                # two-digit e4m3 p for the fp8-nat context matmuls
                p8 = p8s[b]
                nc.vector.tensor_copy(p8[:, sl, 0], p16[:, sl])
                nc.vector.tensor_sub(p8[:, sl, 1], p16[:, sl], p8[:, sl, 0])
            if b == 0 and "p16" in dbg:
                nc.sync.dma_start(dbg["p16"], p16)
                nc.sync.dma_start(dbg["p8"], p8)
                sc_sb = small.tile([128, LTOT], F32, name="sc_sb", tag="dbg")
                nc.vector.tensor_copy(sc_sb, sc_ps[b])
                nc.sync.dma_start(dbg["sc"], sc_sb)

        ctx_pss = {}
        ctx_chunks_done = {b: 0 for b in range(b_loc)}

        def emit_ctx(b, c2):
            p16 = p16s[b]
            p8 = p8s.get(b)
            if b not in ctx_pss:
                ctx_pss[b] = psum_cx.tile([128, OT], F32, name="ctx_ps", tag="cx")
                first = True
            else:
                first = False
            ctx_ps = ctx_pss[b]
            ctx_chunks_done[b] += 1
            last_chunk = ctx_chunks_done[b] == n_ch
            natt = unit_nat.pop((b, c2))
            for lt in range(LT):
                for hi in range(OT):
                    last = (last_chunk and lt == LT - 1 and hi == OT - 1)
                    if c2 < n16:
                        movs = [p16[:, c2 * LT + lt : c2 * LT + lt + 1]]
                    else:
                        movs = [p8[:, c2 * LT + lt, 0:1],
                                p8[:, c2 * LT + lt, 1:2]]
                    for di, mov in enumerate(movs):
                        nc.tensor.matmul(
                            ctx_ps[:, hi : hi + 1],
                            natt[:, lt, hi * 128 : (hi + 1) * 128],
                            mov,
                            start=(first and lt == 0 and hi == 0 and di == 0),
                            stop=(last and di == len(movs) - 1),
                            skip_group_check=True,
                        )

        def emit_finalize(b):
            p16 = p16s[b]
            ctx_ps = ctx_pss.pop(b)
            if b == 0 and "ctx" in dbg:
                cx_sb = small.tile([128, OT], F32, name="cx_sb", tag="dbg")
                nc.vector.tensor_copy(cx_sb, ctx_ps)
                nc.sync.dma_start(dbg["ctx"], cx_sb)
            den1 = small.tile([128, 1], F32, name="den1", tag="den1")
            nc.vector.tensor_reduce(
                den1, p16, mybir.AxisListType.X, mybir.AluOpType.add
            )
            # ones-stationary matmul partition-sums AND broadcasts the
            # denominator to all 128 partitions in one ~free op
            den_rep = psum_sm.tile([128, 1], F32, name="den_rep", tag="sm")
            nc.tensor.matmul(den_rep, ones_sq, den1, start=True, stop=True)
            recip_bc = small.tile([128, 1], F32, name="recip_bc", tag="rbc")
            nc.vector.reciprocal(recip_bc, den_rep)
            ctx_sb = small.tile([128, OT], F32, name="ctx_sb", tag="ctx_sb")
            nc.vector.tensor_copy(ctx_sb, ctx_ps)
            ctxT_ps = psum_sm.tile([OT, 128], F32, name="ctxT_ps", tag="sm")
            nc.tensor.transpose(ctxT_ps, ctx_sb, id128)
            out_row = small.tile([OT, 128], F32, name="out_row", tag="orow")
            nc.vector.tensor_scalar_mul(out_row, ctxT_ps, recip_bc[0:OT, :])
            nc.sync.dma_start(out_ap[b : b + 1, :], out_row)

        for u, (b, c) in enumerate(units):
            if u not in enct_cache:
                load_enct(u)
            enct8 = enct_cache.pop(u)
            if u not in nat_cache:
                load_nat(u)
            unit_nat[(b, c)] = nat_cache.pop(u)
            ea, na = TUNE["enct_ahead"], TUNE["nat_ahead"]
            if u + ea < n_units and u + ea not in enct_cache:
                load_enct(u + ea)
            if u + na < n_units and u + na not in nat_cache:
                load_nat(u + na)
            if b not in sc_ps:
                sc_ps[b] = psum_sc.tile([128, LTOT], F32, name="sc_ps", tag="sc")

            engs = [None] * OT
            for oi in range(OT):
                mm_ps = psum_mm.tile([128, ch], F32, name="mm_ps", tag="mm")
                for half in range(ch // 512):
                    hs = slice(half * 512, (half + 1) * 512)
                    for q in range(HT // 2):
                        nc.tensor.matmul(
                            mm_ps[:, hs],
                            w8_all[:, oi, 2 * q : 2 * q + 2, :],
                            enct8[:, 2 * q : 2 * q + 2, hs],
                            start=(q == 0),
                            stop=(q == HT // 2 - 1),
                            perf_mode=DR,
                        )
                if u == 0 and oi > 0:
                    emit_bias(oi)
                eng = eng_pool.tile([128, ch], F16, name="eng", tag="eng")
                nc.scalar.activation(
                    eng,
                    mm_ps,
                    AF.Tanh,
                    bias=bias_sb[:, oi * b_loc + b : oi * b_loc + b + 1],
                    scale=1.0,
                )
                engs[oi] = eng
                if u == 0 and oi == 0 and "eng00" in dbg:
                    nc.sync.dma_start(dbg["eng00"], eng)
                if u == 0 and oi == OT - 1 and "bias" in dbg:
                    nc.sync.dma_start(dbg["bias"], bias_sb)
                for cb in hooks.pop((u, oi), ()):
                    cb()
                if oi >= 2:
                    emit_score(b, c, oi - 2, engs)

            # defer this unit's last two score columns + tails into the next
            # unit's stream so PE never blocks on the ACT queue
            def _mk(fn, *args):
                return lambda: fn(*args)

            chunks_done[b] += 1
            batch_complete = chunks_done[b] == n_ch
            if u + 1 < n_units:
                hooks.setdefault((u + 1, 0), []).append(
                    _mk(emit_score, b, c, OT - 2, engs)
                )
                nxt1 = hooks.setdefault((u + 1, TUNE["exp_hook"]), [])
                nxt1.append(_mk(emit_score, b, c, OT - 1, engs))
                nxt1.append(_mk(emit_exp_psplit, b, c))
                hooks.setdefault((u + 1, TUNE["ctx_hook"]), []).append(
                    _mk(emit_ctx, b, c))
                if batch_complete:
                    hooks.setdefault((u + 1, TUNE["ctx_hook"] + 1), []).append(
                        _mk(emit_finalize, b)
                    )
            else:
                emit_score(b, c, OT - 2, engs)
                emit_score(b, c, OT - 1, engs)
                emit_exp_psplit(b, c)
                emit_ctx(b, c)
                emit_finalize(b)


def build_bass(b_loc=B_LOC, l_total=L, enable_asserts=False, debug_taps=False):
    """Build + schedule + compile the Bass module. Returns (nc, out_name)."""
    nc = bacc.Bacc(
        "TRN2",
        target_bir_lowering=False,
        debug=False,
        enable_asserts=enable_asserts,
        num_devices=N_CORES,
    )
    ins = {
        "hidden": nc.dram_tensor("hidden", [b_loc, H], F32, kind="ExternalInput").ap(),
        "enc8": nc.dram_tensor(
            "enc8", [b_loc, H, l_total], FP8, kind="ExternalInput"
        ).ap(),
        "nat16": nc.dram_tensor(
            "nat16", [_l16(l_total), b_loc, H], F16, kind="ExternalInput"
        ).ap(),
        "w8": nc.dram_tensor(
            "w8", [128, OT, HT, 128], FP8, kind="ExternalInput"
        ).ap(),
        "wh16": nc.dram_tensor(
            "wh16", [128, OT, HT, 128], F16, kind="ExternalInput"
        ).ap(),
        "attn_b": nc.dram_tensor("attn_b", [H], F32, kind="ExternalInput").ap(),
        "v": nc.dram_tensor("v", [H], F32, kind="ExternalInput").ap(),
    }
    if l_total > _l16(l_total):
        ins["nat8"] = nc.dram_tensor(
            "nat8", [l_total - _l16(l_total), b_loc, H], FP8, kind="ExternalInput"
        ).ap()
    out = nc.dram_tensor("ctx_out", [b_loc, H], F32, kind="ExternalOutput").ap()
    dbg = {}
    if debug_taps:
        ch = min(1024, l_total)
        LTOT = l_total // 128
        dbg = {
            "eng00": nc.dram_tensor("dbg_eng00", [128, ch], F16, kind="ExternalOutput").ap(),
            "bias": nc.dram_tensor("dbg_bias", [128, OT * b_loc], F32, kind="ExternalOutput").ap(),
            "p16": nc.dram_tensor("dbg_p16", [128, LTOT], F16, kind="ExternalOutput").ap(),
            "p8": nc.dram_tensor("dbg_p8", [128, LTOT, 2], FP8, kind="ExternalOutput").ap(),
            "sc": nc.dram_tensor("dbg_sc", [128, LTOT], F32, kind="ExternalOutput").ap(),
            "ctx": nc.dram_tensor("dbg_ctx", [128, OT], F32, kind="ExternalOutput").ap(),
        }
    with tile.TileContext(nc) as tc:
        build_attn_kernel(tc, out, ins, b_loc=b_loc, l_total=l_total, dbg=dbg)
    nc.compile()
    return nc, "ctx_out"


@functools.cache
def _built():
    return build_bass()


def kernel(hidden, encoder_outputs, attn_w, attn_b, v):
    """Full-input entry point: shard over batch, run 8 cores, gather."""
    global LAST_RESULTS
    from concourse.bass_utils import run_bass_kernel_spmd

    hidden = np.ascontiguousarray(np.asarray(hidden, dtype=np.float32))
    encoder_outputs = np.ascontiguousarray(
        np.asarray(encoder_outputs, dtype=np.float32)
    )
    attn_w = np.ascontiguousarray(np.asarray(attn_w, dtype=np.float32))
    attn_b = np.ascontiguousarray(np.asarray(attn_b, dtype=np.float32))
    v = np.ascontiguousarray(np.asarray(v, dtype=np.float32))

    prep = prep_host(hidden, encoder_outputs, attn_w, attn_b, v)

    nc, out_name = _built()
    in_maps = []
    for cidx in range(N_CORES):
        bs = slice(cidx * B_LOC, (cidx + 1) * B_LOC)
        in_maps.append(
            {
                "hidden": prep["hidden"][bs],
                "enc8": np.ascontiguousarray(prep["enc8"][bs]),
                "nat16": np.ascontiguousarray(prep["nat16"][:, bs, :]),
                **({"nat8": np.ascontiguousarray(prep["nat8"][:, bs, :])}
                   if "nat8" in prep else {}),
                "w8": prep["w8"],
                "wh16": prep["wh16"],
                "attn_b": prep["attn_b"],
                "v": prep["v"],
            }
        )
    res = run_bass_kernel_spmd(
        nc,
        in_maps,
        core_ids=list(range(N_CORES)),
        trace=bool(os.environ.get("BASS_TRACE")),
    )
    LAST_RESULTS = res
    out = np.concatenate(
        [res.results[cidx][out_name] for cidx in range(N_CORES)], axis=0
    )
    return out[None, :, :].astype(np.float32)
